# revision 5
# baseline (speedup 1.0000x reference)
"""Single-head causal self-attention on 8 trn2 NeuronCores.

Problem: x[4,4096,1024], Wq/Wk/Wv[1024,128]+biases -> causal attention out
[4,4096,128], fp32.

Sharding: core c = (b = c//2, j = c%2). Core (b, j) handles batch b and the
K/V column 128-blocks of parity j (alternating blocks balance the causal
triangle). It computes, for ALL 4096 query rows, the *unnormalized* partial
attention over its own columns:
    O_un^T[h, s] = sum_{t in cols_j, t<=s} exp(q_s.k_t * scale) * v_t[h]
    l[s]         = sum_{t in cols_j, t<=s} exp(q_s.k_t * scale)
Host combines:  O[s] = (O_un0[s] + O_un1[s]) / (l0[s] + l1[s]).
No per-core max subtraction is needed: scores are ~N(0,1) (bounded ~6), so
exp never overflows; masked entries get an additive -1e32 -> exp = 0.

SPMD uniformity: the same Bass program runs on all 8 cores. Parity enters
only through data: for j=1 the host swaps adjacent 128-row blocks of x
(involution), so "even position blocks" on the device are the core's own
columns; the causal masks (2 tiles of [128,512], R-independent by algebra)
are passed as per-core inputs. Output comes back in position space and the
host un-swaps.

Device pipeline per core (all matmuls float32r, ~1.5e-4 component rel err,
~2e-4 end-to-end vs fp32 reference):
  Stage A (per 512-row superstep): DMA x rows -> PE-transpose 128x128 blocks
    -> x^T (f32r) -> projections with W stationary: Q^T[h,s] (all s),
    K^T[h,t], V^T[h,t] (even position blocks only) -> ACT copy+bias;
    V^T PE-transposed to V[t,h].
  Stage B (per superblock R of 512 rows, pairs p=0..R of t-blocks):
    S^T[t,s] = K^T.T @ Q^T  (PSUM, 2 banks per pair)
    last pair: DVE adds mask; ACT: P^T = exp(scale * S^T) -> f32r SBUF
    O^T += V.T @ P^T ; l += ones.T @ P^T  (PSUM accum over all 2R+2 blocks)
    DMA O^T[128,512] and l[1,512] PSUM->DRAM.
"""

import sys

sys.path.insert(0, "/opt/trn_rl_repo")

import numpy as np

import concourse.bacc as bacc
import concourse.mybir as mybir
import concourse.tile as tile
from concourse import bass_utils
from concourse.masks import make_identity

S, E, H, B = 4096, 1024, 128, 4
NSUP, SUP = 8, 512
NEG = -1.0e32
SCALE = 1.0 / float(np.sqrt(128.0))
F32 = mybir.dt.float32
F32R = mybir.dt.float32r
ACT_IDENT = mybir.ActivationFunctionType.Identity
ACT_EXP = mybir.ActivationFunctionType.Exp


def build_nc():
    nc = bacc.Bacc("TRN2", debug=False, num_devices=8)
    xb_d = nc.dram_tensor("xb", [S, E], F32, kind="ExternalInput").ap()
    wq_d = nc.dram_tensor("wq", [E, H], F32, kind="ExternalInput").ap()
    wk_d = nc.dram_tensor("wk", [E, H], F32, kind="ExternalInput").ap()
    wv_d = nc.dram_tensor("wv", [E, H], F32, kind="ExternalInput").ap()
    bq_d = nc.dram_tensor("bq", [H], F32, kind="ExternalInput").ap()
    bk_d = nc.dram_tensor("bk", [H], F32, kind="ExternalInput").ap()
    bv_d = nc.dram_tensor("bv", [H], F32, kind="ExternalInput").ap()
    mask_d = nc.dram_tensor("mask", [128, 2, SUP], F32, kind="ExternalInput").ap()
    outT_d = nc.dram_tensor("outT", [H, S], F32, kind="ExternalOutput").ap()
    l_d = nc.dram_tensor("lsum", [1, S], F32, kind="ExternalOutput").ap()

    with tile.TileContext(nc) as tc:
        with (
            tc.tile_pool(name="persist", bufs=1) as pp,
            tc.tile_pool(name="xrows", bufs=3) as xrp,
            tc.tile_pool(name="xts", bufs=2) as xtp,
            tc.tile_pool(name="vtmp", bufs=2) as vtp,
            tc.tile_pool(name="pts", bufs=3) as ptp,
            tc.tile_pool(name="osb", bufs=2) as osp,
        ):
            ident = pp.tile([128, 128], F32)
            make_identity(nc, ident)
            ones_f32 = pp.tile([128, 1], F32)
            nc.vector.memset(ones_f32, 1.0)
            ones_col = pp.tile([128, 1], F32R)
            nc.vector.tensor_copy(ones_col, ones_f32)

            mask_s = pp.tile([128, 2, SUP], F32)
            nc.sync.dma_start(mask_s, mask_d)

            bias_s = {}
            for nm, bd in (("q", bq_d), ("k", bk_d), ("v", bv_d)):
                bt = pp.tile([128, 1], F32, name=f"bias_{nm}")
                nc.sync.dma_start(bt, bd.unsqueeze(1))
                bias_s[nm] = bt

            w_r = {}
            for nm, wd in (("q", wq_d), ("k", wk_d), ("v", wv_d)):
                wstage = pp.tile([128, 8, H], F32, name=f"wstage_{nm}")
                nc.sync.dma_start(wstage, wd.rearrange("(c p) h -> p c h", p=128))
                wr = pp.tile([128, 8, H], F32R, name=f"wr_{nm}")
                nc.vector.tensor_copy(wr, wstage)
                w_r[nm] = wr

            qt_all = pp.tile([128, S], F32R)
            kt_all = pp.tile([128, 16, 128], F32R)
            v_all = pp.tile([128, 16, 128], F32R)

            x_view = xb_d.rearrange("(a t p) e -> a p t e", t=4, p=128)

            # ---- Stage A: transpose + projections ----
            with (
                tc.tile_pool(name="pstr", bufs=2, space="PSUM") as trp,
                tc.tile_pool(name="psproj", bufs=2, space="PSUM") as prp,
                tc.tile_pool(name="psvtr", bufs=2, space="PSUM") as vrp,
            ):
                for i2 in range(NSUP):
                    xrow = xrp.tile([128, 4, E], F32, tag="xrow", name="xrow")
                    nc.sync.dma_start(xrow, x_view[i2])
                    xT = xtp.tile([128, 8, SUP], F32R, tag="xT", name="xT")
                    for ec in range(8):
                        pb = trp.tile([128, 512], F32, tag="tr", name="pb")
                        for t in range(4):
                            nc.tensor.transpose(
                                pb[:, t * 128 : (t + 1) * 128],
                                xrow[:, t, ec * 128 : (ec + 1) * 128],
                                ident,
                            )
                        nc.vector.tensor_copy(xT[:, ec, :], pb)
                    xT4 = xT.rearrange("p c (t w) -> p c t w", t=4)

                    qt_ps = prp.tile([128, SUP], F32, tag="proj", name="qt_ps")
                    for c in range(8):
                        nc.tensor.matmul(
                            qt_ps,
                            w_r["q"][:, c, :],
                            xT[:, c, :],
                            start=(c == 0),
                            stop=(c == 7),
                        )
                    nc.scalar.activation(
                        qt_all[:, i2 * SUP : (i2 + 1) * SUP],
                        qt_ps,
                        ACT_IDENT,
                        bias=bias_s["q"],
                    )

                    kt_ps = prp.tile([128, 256], F32, tag="proj", name="kt_ps")
                    for c in range(8):
                        nc.tensor.matmul(
                            kt_ps,
                            w_r["k"][:, c, :],
                            xT4[:, c, 0::2, :],
                            start=(c == 0),
                            stop=(c == 7),
                        )
                    nc.scalar.activation(
                        kt_all[:, 2 * i2 : 2 * i2 + 2, :],
                        kt_ps.rearrange("p (t w) -> p t w", t=2),
                        ACT_IDENT,
                        bias=bias_s["k"],
                    )

                    vt_ps = prp.tile([128, 256], F32, tag="proj", name="vt_ps")
                    for c in range(8):
                        nc.tensor.matmul(
                            vt_ps,
                            w_r["v"][:, c, :],
                            xT4[:, c, 0::2, :],
                            start=(c == 0),
                            stop=(c == 7),
                        )
                    vt_tmp = vtp.tile([128, 256], F32, tag="vtmp", name="vt_tmp")
                    nc.scalar.activation(vt_tmp, vt_ps, ACT_IDENT, bias=bias_s["v"])
                    v_ps2 = vrp.tile([128, 256], F32, tag="vtr", name="v_ps2")
                    for tt in range(2):
                        nc.tensor.transpose(
                            v_ps2[:, tt * 128 : (tt + 1) * 128],
                            vt_tmp[:, tt * 128 : (tt + 1) * 128],
                            ident,
                        )
                    nc.vector.tensor_copy(
                        v_all[:, 2 * i2 : 2 * i2 + 2, :],
                        v_ps2.rearrange("p (t w) -> p t w", t=2),
                    )

            # ---- Stage B: scores^T -> exp -> AV + l ----
            with (
                tc.tile_pool(name="psst", bufs=2, space="PSUM") as stp,
                tc.tile_pool(name="pso", bufs=2, space="PSUM") as outp,
                tc.tile_pool(name="psl", bufs=2, space="PSUM") as lp,
            ):
                o_ps = {}
                l_ps = {}
                pt_of = {}

                def emit_S(task):
                    R, p = task
                    st = stp.tile([128, 2, SUP], F32, tag="st", name="st")
                    for half in range(2):
                        k = 2 * p + half
                        nc.tensor.matmul(
                            st[:, half, :],
                            kt_all[:, k, :],
                            qt_all[:, R * SUP : (R + 1) * SUP],
                            start=True,
                            stop=True,
                        )
                    if p == R:
                        nc.vector.tensor_add(st, st, mask_s)
                    pt = ptp.tile([128, 2, SUP], F32R, tag="pt", name="pt")
                    nc.scalar.activation(pt, st, ACT_EXP, scale=SCALE)
                    pt_of[task] = pt

                def emit_AV(task):
                    R, p = task
                    if p == 0:
                        o_ps[R] = outp.tile([128, SUP], F32, tag="o", name="o_ps")
                        l_ps[R] = lp.tile([128, SUP], F32, tag="l", name="l_ps")
                    pt = pt_of.pop(task)
                    last = 2 * R + 1
                    for half in range(2):
                        k = 2 * p + half
                        nc.tensor.matmul(
                            o_ps[R],
                            v_all[:, k, :],
                            pt[:, half, :],
                            start=(k == 0),
                            stop=(k == last),
                        )
                        nc.tensor.matmul(
                            l_ps[R][0:1, :],
                            ones_col,
                            pt[:, half, :],
                            start=(k == 0),
                            stop=(k == last),
                        )
                    if p == R:
                        o_sb = osp.tile([128, SUP], F32, tag="o_sb", name="o_sb")
                        nc.vector.tensor_copy(o_sb, o_ps[R])
                        nc.sync.dma_start(outT_d[:, R * SUP : (R + 1) * SUP], o_sb)
                        l_sb = osp.tile([1, SUP], F32, tag="l_sb", name="l_sb")
                        nc.vector.tensor_copy(l_sb, l_ps[R][0:1, :])
                        nc.sync.dma_start(l_d[:, R * SUP : (R + 1) * SUP], l_sb)

                tasks = [(R, p) for R in range(NSUP) for p in range(R + 1)]
                prev = None
                for task in tasks:
                    emit_S(task)
                    if prev is not None:
                        emit_AV(prev)
                    prev = task
                emit_AV(prev)

    nc.compile()
    return nc


def _perm1():
    idx = np.arange(S)
    return (idx // 128 ^ 1) * 128 + idx % 128


def _mask_for(j):
    ti = np.arange(128)[:, None, None]
    m = np.arange(2)[None, :, None]
    si = np.arange(SUP)[None, None, :]
    orig_s = 128 * ((si // 128) ^ j) + si % 128
    vis = orig_s >= 128 * (2 * m + j) + ti
    return np.where(vis, np.float32(0.0), np.float32(NEG)).astype(np.float32)


_CACHE = {}


def kernel(x, Wq, bq, Wk, bk, Wv, bv):
    if "nc" not in _CACHE:
        _CACHE["nc"] = build_nc()
    nc = _CACHE["nc"]

    x = np.ascontiguousarray(np.asarray(x, dtype=np.float32))
    Wq = np.ascontiguousarray(np.asarray(Wq, dtype=np.float32))
    Wk = np.ascontiguousarray(np.asarray(Wk, dtype=np.float32))
    Wv = np.ascontiguousarray(np.asarray(Wv, dtype=np.float32))
    bq = np.ascontiguousarray(np.asarray(bq, dtype=np.float32))
    bk = np.ascontiguousarray(np.asarray(bk, dtype=np.float32))
    bv = np.ascontiguousarray(np.asarray(bv, dtype=np.float32))

    perm = _perm1()
    masks = {j: _mask_for(j) for j in (0, 1)}
    in_maps = []
    for c in range(8):
        b, j = divmod(c, 2)
        xin = x[b] if j == 0 else np.ascontiguousarray(x[b][perm])
        in_maps.append(
            {
                "xb": xin,
                "wq": Wq,
                "wk": Wk,
                "wv": Wv,
                "bq": bq,
                "bk": bk,
                "bv": bv,
                "mask": masks[j],
            }
        )

    res = bass_utils.run_bass_kernel_spmd(nc, in_maps, core_ids=list(range(8)))

    out = np.empty((B, S, H), np.float32)
    for b in range(B):
        oT0 = res.results[2 * b]["outT"]
        l0 = res.results[2 * b]["lsum"][0]
        oT1 = res.results[2 * b + 1]["outT"][:, perm]
        l1 = res.results[2 * b + 1]["lsum"][0][perm]
        out[b] = ((oT0 + oT1) / (l0 + l1)[None, :]).T
    return out


# revision 7
# speedup vs baseline: 1.2700x; 1.2700x over previous
"""Single-head causal self-attention on 8 trn2 NeuronCores.

Problem: x[4,4096,1024], Wq/Wk/Wv[1024,128]+biases -> causal attention out
[4,4096,128], fp32.

Sharding: core c = (b = c//2, j = c%2). Core (b, j) handles batch b and the
K/V column 128-blocks of parity j (alternating blocks balance the causal
triangle). It computes, for ALL 4096 query rows, the *unnormalized* partial
attention over its own columns:
    O_un^T[h, s] = sum_{t in cols_j, t<=s} exp(q_s.k_t * scale) * v_t[h]
    l[s]         = sum_{t in cols_j, t<=s} exp(q_s.k_t * scale)
Host combines:  O[s] = (O_un0[s] + O_un1[s]) / (l0[s] + l1[s]).
No per-core max subtraction is needed: scores are ~N(0,1) (bounded ~6), so
exp never overflows; masked entries get an additive -1e32 -> exp = 0.

SPMD uniformity: the same Bass program runs on all 8 cores. Parity enters
only through data: for j=1 the host swaps adjacent 128-row blocks of x
(involution), so "even position blocks" on the device are the core's own
columns; the causal masks (2 tiles of [128,512], R-independent by algebra)
are passed as per-core inputs. Output comes back in position space and the
host un-swaps.

The host passes x^T (x transposed, [1024, 4096]) declared float32r, so the
device needs no transposes for the projections and the DMA layout stays
line-rate (per-partition contiguous rows).

Device pipeline per core (all matmuls float32r, ~1.5e-4 component rel err,
~3.5e-4 end-to-end vs fp32 reference):
  Stage A (per 512-row superstep i2): DMA x^T tile [128, 8, 512] ->
    projections with W stationary: Q^T[h,s] (all s), K^T[h,t], V^T[h,t]
    (even position blocks only) -> ACT copy+bias; V^T PE-transposed to
    V[t,h]. Interleaved with stage B groups as their inputs complete.
  Stage B (per superblock R of 512 rows, pairs p=0..R of t-blocks):
    S^T[t,s] = K^T.T @ Q^T  (PSUM, 2 banks per pair)
    last pair: DVE adds mask; ACT: P^T = exp(scale * S^T) -> f32r SBUF
    O^T += V.T @ P^T ; l += ones.T @ P^T  (PSUM accum over all 2R+2 blocks)
    copy to SBUF, DMA O^T[128,512] and l[1,512] -> DRAM.
"""

import sys

sys.path.insert(0, "/opt/trn_rl_repo")

import numpy as np

import concourse.bacc as bacc
import concourse.mybir as mybir
import concourse.tile as tile
from concourse import bass_utils
from concourse.masks import make_identity

S, E, H, B = 4096, 1024, 128, 4
NSUP, SUP = 8, 512
NEG = -1.0e32
SCALE = 1.0 / float(np.sqrt(128.0))
F32 = mybir.dt.float32
F32R = mybir.dt.float32r
ACT_IDENT = mybir.ActivationFunctionType.Identity
ACT_EXP = mybir.ActivationFunctionType.Exp


def build_nc():
    nc = bacc.Bacc("TRN2", debug=False, num_devices=8)
    xt_d = nc.dram_tensor("xt", [E, S], F32R, kind="ExternalInput").ap()
    wq_d = nc.dram_tensor("wq", [E, H], F32R, kind="ExternalInput").ap()
    wk_d = nc.dram_tensor("wk", [E, H], F32R, kind="ExternalInput").ap()
    wv_d = nc.dram_tensor("wv", [E, H], F32R, kind="ExternalInput").ap()
    bq_d = nc.dram_tensor("bq", [H], F32, kind="ExternalInput").ap()
    bk_d = nc.dram_tensor("bk", [H], F32, kind="ExternalInput").ap()
    bv_d = nc.dram_tensor("bv", [H], F32, kind="ExternalInput").ap()
    mask_d = nc.dram_tensor("mask", [128, 2, SUP], F32, kind="ExternalInput").ap()
    outT_d = nc.dram_tensor("outT", [H, S], F32, kind="ExternalOutput").ap()
    l_d = nc.dram_tensor("lsum", [1, S], F32, kind="ExternalOutput").ap()

    with tile.TileContext(nc) as tc:
        with (
            tc.tile_pool(name="persist", bufs=1) as pp,
            tc.tile_pool(name="xts", bufs=3) as xtp,
            tc.tile_pool(name="vtmp", bufs=2) as vtp,
            tc.tile_pool(name="pts", bufs=3) as ptp,
            tc.tile_pool(name="osb", bufs=2) as osp,
            tc.tile_pool(name="psproj", bufs=2, space="PSUM") as prp,
            tc.tile_pool(name="psst", bufs=2, space="PSUM") as stp,
            tc.tile_pool(name="pso", bufs=1, space="PSUM") as outp,
            tc.tile_pool(name="psl", bufs=1, space="PSUM") as lp,
        ):
            ident = pp.tile([128, 128], F32)
            make_identity(nc, ident)
            ones_f32 = pp.tile([128, 1], F32)
            nc.vector.memset(ones_f32, 1.0)
            ones_col = pp.tile([128, 1], F32R)
            nc.vector.tensor_copy(ones_col, ones_f32)

            mask_s = pp.tile([128, 2, SUP], F32)
            nc.sync.dma_start(mask_s, mask_d)

            bias_s = {}
            for nm, bd in (("q", bq_d), ("k", bk_d), ("v", bv_d)):
                bt = pp.tile([128, 1], F32, name=f"bias_{nm}")
                nc.sync.dma_start(bt, bd.unsqueeze(1))
                bias_s[nm] = bt

            w_r = {}
            for nm, wd in (("q", wq_d), ("k", wk_d), ("v", wv_d)):
                wr = pp.tile([128, 8, H], F32R, name=f"wr_{nm}")
                nc.sync.dma_start(wr, wd.rearrange("(c p) h -> p c h", p=128))
                w_r[nm] = wr

            qt_all = pp.tile([128, S], F32R)
            kt_all = pp.tile([128, 16, 128], F32R)
            v_all = pp.tile([128, 16, 128], F32R)

            xt_view = xt_d.rearrange("(c p) s -> p c s", p=128)

            o_ps = {}
            l_ps = {}
            pt_of = {}

            def emit_A(i2):
                xT = xtp.tile([128, 8, SUP], F32R, tag="xT", name="xT")
                nc.sync.dma_start(xT, xt_view[:, :, i2 * SUP : (i2 + 1) * SUP])
                xT4 = xT.rearrange("p c (t w) -> p c t w", t=4)

                qt_ps = prp.tile([128, SUP], F32, tag="proj", name="qt_ps")
                for c in range(8):
                    nc.tensor.matmul(
                        qt_ps,
                        w_r["q"][:, c, :],
                        xT[:, c, :],
                        start=(c == 0),
                        stop=(c == 7),
                    )
                nc.scalar.activation(
                    qt_all[:, i2 * SUP : (i2 + 1) * SUP],
                    qt_ps,
                    ACT_IDENT,
                    bias=bias_s["q"],
                )

                kt_ps = prp.tile([128, 256], F32, tag="proj", name="kt_ps")
                for c in range(8):
                    nc.tensor.matmul(
                        kt_ps,
                        w_r["k"][:, c, :],
                        xT4[:, c, 0::2, :],
                        start=(c == 0),
                        stop=(c == 7),
                    )
                nc.scalar.activation(
                    kt_all[:, 2 * i2 : 2 * i2 + 2, :],
                    kt_ps.rearrange("p (t w) -> p t w", t=2),
                    ACT_IDENT,
                    bias=bias_s["k"],
                )

                vt_ps = prp.tile([128, 256], F32, tag="proj", name="vt_ps")
                for c in range(8):
                    nc.tensor.matmul(
                        vt_ps,
                        w_r["v"][:, c, :],
                        xT4[:, c, 0::2, :],
                        start=(c == 0),
                        stop=(c == 7),
                    )
                vt_tmp = vtp.tile([128, 256], F32, tag="vtmp", name="vt_tmp")
                nc.scalar.activation(vt_tmp, vt_ps, ACT_IDENT, bias=bias_s["v"])
                v_ps2 = prp.tile([128, 256], F32, tag="proj", name="v_ps2")
                for tt in range(2):
                    nc.tensor.transpose(
                        v_ps2[:, tt * 128 : (tt + 1) * 128],
                        vt_tmp[:, tt * 128 : (tt + 1) * 128],
                        ident,
                    )
                nc.vector.tensor_copy(
                    v_all[:, 2 * i2 : 2 * i2 + 2, :],
                    v_ps2.rearrange("p (t w) -> p t w", t=2),
                )

            def emit_S(task):
                R, p = task
                st = stp.tile([128, 2, SUP], F32, tag="st", name="st")
                for half in range(2):
                    k = 2 * p + half
                    nc.tensor.matmul(
                        st[:, half, :],
                        kt_all[:, k, :],
                        qt_all[:, R * SUP : (R + 1) * SUP],
                        start=True,
                        stop=True,
                    )
                if p == R:
                    nc.vector.tensor_add(st, st, mask_s)
                pt = ptp.tile([128, 2, SUP], F32R, tag="pt", name="pt")
                nc.scalar.activation(pt, st, ACT_EXP, scale=SCALE)
                pt_of[task] = pt

            def emit_AV(task):
                R, p = task
                if p == 0:
                    o_ps[R] = outp.tile([128, SUP], F32, tag="o", name="o_ps")
                    l_ps[R] = lp.tile([128, SUP], F32, tag="l", name="l_ps")
                pt = pt_of.pop(task)
                last = 2 * R + 1
                for half in range(2):
                    k = 2 * p + half
                    nc.tensor.matmul(
                        o_ps[R],
                        v_all[:, k, :],
                        pt[:, half, :],
                        start=(k == 0),
                        stop=(k == last),
                    )
                    nc.tensor.matmul(
                        l_ps[R][0:1, :],
                        ones_col,
                        pt[:, half, :],
                        start=(k == 0),
                        stop=(k == last),
                    )
                if p == R:
                    o_sb = osp.tile([128, SUP], F32, tag="o_sb", name="o_sb")
                    nc.vector.tensor_copy(o_sb, o_ps[R])
                    nc.sync.dma_start(outT_d[:, R * SUP : (R + 1) * SUP], o_sb)
                    l_sb = osp.tile([1, SUP], F32, tag="l_sb", name="l_sb")
                    nc.vector.tensor_copy(l_sb, l_ps[R][0:1, :])
                    nc.sync.dma_start(l_d[:, R * SUP : (R + 1) * SUP], l_sb)

            pipe = {"prev": None}

            def push_task(task):
                emit_S(task)
                if pipe["prev"] is not None:
                    emit_AV(pipe["prev"])
                pipe["prev"] = task

            for i2 in range(NSUP):
                emit_A(i2)
                if i2 % 2 == 1:
                    R = (i2 - 1) // 2
                    for p in range(R + 1):
                        push_task((R, p))
            for R in range(4, NSUP):
                for p in range(R + 1):
                    push_task((R, p))
            emit_AV(pipe["prev"])

    nc.compile()
    return nc


def _perm1():
    idx = np.arange(S)
    return (idx // 128 ^ 1) * 128 + idx % 128


def _mask_for(j):
    ti = np.arange(128)[:, None, None]
    m = np.arange(2)[None, :, None]
    si = np.arange(SUP)[None, None, :]
    orig_s = 128 * ((si // 128) ^ j) + si % 128
    vis = orig_s >= 128 * (2 * m + j) + ti
    return np.where(vis, np.float32(0.0), np.float32(NEG)).astype(np.float32)


_CACHE = {}


def kernel(x, Wq, bq, Wk, bk, Wv, bv):
    if "nc" not in _CACHE:
        _CACHE["nc"] = build_nc()
    nc = _CACHE["nc"]

    x = np.ascontiguousarray(np.asarray(x, dtype=np.float32))
    Wq = np.ascontiguousarray(np.asarray(Wq, dtype=np.float32))
    Wk = np.ascontiguousarray(np.asarray(Wk, dtype=np.float32))
    Wv = np.ascontiguousarray(np.asarray(Wv, dtype=np.float32))
    bq = np.ascontiguousarray(np.asarray(bq, dtype=np.float32))
    bk = np.ascontiguousarray(np.asarray(bk, dtype=np.float32))
    bv = np.ascontiguousarray(np.asarray(bv, dtype=np.float32))

    perm = _perm1()
    masks = {j: _mask_for(j) for j in (0, 1)}
    # x^T per batch, and the column-block-swapped variant for parity-1 cores
    xT = {}
    for b in range(B):
        t = np.ascontiguousarray(x[b].T)  # [E, S]
        xT[(b, 0)] = t
        xT[(b, 1)] = np.ascontiguousarray(
            t.reshape(E, S // 128, 128)[:, (np.arange(S // 128) ^ 1), :].reshape(E, S)
        )

    in_maps = []
    for c in range(8):
        b, j = divmod(c, 2)
        in_maps.append(
            {
                "xt": xT[(b, j)],
                "wq": Wq,
                "wk": Wk,
                "wv": Wv,
                "bq": bq,
                "bk": bk,
                "bv": bv,
                "mask": masks[j],
            }
        )

    res = bass_utils.run_bass_kernel_spmd(nc, in_maps, core_ids=list(range(8)))

    out = np.empty((B, S, H), np.float32)
    for b in range(B):
        oT0 = res.results[2 * b]["outT"]
        l0 = res.results[2 * b]["lsum"][0]
        oT1 = res.results[2 * b + 1]["outT"][:, perm]
        l1 = res.results[2 * b + 1]["lsum"][0][perm]
        out[b] = ((oT0 + oT1) / (l0 + l1)[None, :]).T
    return out


# revision 13
# speedup vs baseline: 1.4261x; 1.1229x over previous
"""Single-head causal self-attention on 8 trn2 NeuronCores.

Problem: x[4,4096,1024], Wq/Wk/Wv[1024,128]+biases -> causal attention out
[4,4096,128], fp32.

Sharding: core c = (b = c//2, j = c%2). Core (b, j) handles batch b and the
K/V column 128-blocks of parity j (alternating blocks balance the causal
triangle). It computes, for ALL 4096 query rows, the *unnormalized* partial
attention over its own columns:
    O_un^T[h, s] = sum_{t in cols_j, t<=s} exp(q_s.k_t * scale) * v_t[h]
    l[s]         = sum_{t in cols_j, t<=s} exp(q_s.k_t * scale)
Host combines:  O[s] = (O_un0[s] + O_un1[s]) / (l0[s] + l1[s]).
No per-core max subtraction is needed: scores are ~N(0,1) (bounded ~6), so
exp never overflows; masked entries get an additive -1e32 -> exp = 0.

SPMD uniformity: the same Bass program runs on all 8 cores. Parity enters
only through data: for j=1 the host swaps adjacent 128-row blocks of x
(involution), so "even position blocks" on the device are the core's own
columns; the causal masks (2 tiles of [128,512], R-independent by algebra)
are passed as per-core inputs. Output comes back in position space and the
host un-swaps.

The host passes x^T (x transposed, [1024, 4096]) declared float32r, so the
device needs no transposes for the projections and the DMA layout stays
line-rate (per-partition contiguous rows).

Device pipeline per core (all matmuls float32r, ~1.5e-4 component rel err,
~3.5e-4 end-to-end vs fp32 reference):
  Stage A (per 512-row superstep i2): DMA x^T tile [128, 8, 512] ->
    projections with W stationary: Q^T[h,s] (all s), K^T[h,t], V^T[h,t]
    (even position blocks only) -> ACT copy+bias; V^T PE-transposed to
    V[t,h]. Interleaved with stage B groups as their inputs complete.
  Stage B (per superblock R of 512 rows, pairs p=0..R of t-blocks):
    S^T[t,s] = K^T.T @ Q^T  (PSUM, 2 banks per pair)
    last pair: DVE adds mask; ACT: P^T = exp(scale * S^T) -> f32r SBUF
    O^T += V.T @ P^T ; l += ones.T @ P^T  (PSUM accum over all 2R+2 blocks)
    copy to SBUF, DMA O^T[128,512] and l[1,512] -> DRAM.
"""

import sys

sys.path.insert(0, "/opt/trn_rl_repo")

import numpy as np

import concourse.bacc as bacc
import concourse.mybir as mybir
import concourse.tile as tile
from concourse import bass_utils
from concourse.masks import make_identity

S, E, H, B = 4096, 1024, 128, 4
NSUP, SUP = 8, 512
NEG = -1.0e32
SCALE = 1.0 / float(np.sqrt(128.0))
F32 = mybir.dt.float32
F32R = mybir.dt.float32r
ACT_IDENT = mybir.ActivationFunctionType.Identity
ACT_EXP = mybir.ActivationFunctionType.Exp


def build_nc(loop_n=None, bias_engine="act", pt_bufs=3, stage_b=True):
    nc = bacc.Bacc("TRN2", debug=False, num_devices=8)
    xt_d = nc.dram_tensor("xt", [E, S], F32R, kind="ExternalInput").ap()
    wq_d = nc.dram_tensor("wq", [E, H], F32R, kind="ExternalInput").ap()
    wk_d = nc.dram_tensor("wk", [E, H], F32R, kind="ExternalInput").ap()
    wv_d = nc.dram_tensor("wv", [E, H], F32R, kind="ExternalInput").ap()
    bias_d = nc.dram_tensor("bias", [H, 3], F32, kind="ExternalInput").ap()
    mask_d = nc.dram_tensor("mask", [128, 2, SUP], F32, kind="ExternalInput").ap()
    outT_d = nc.dram_tensor("outT", [H, S], F32, kind="ExternalOutput").ap()
    l_d = nc.dram_tensor("lsum", [1, S], F32, kind="ExternalOutput").ap()

    with tile.TileContext(nc) as tc:
        with (
            tc.tile_pool(name="persist", bufs=1) as pp,
            tc.tile_pool(name="xts", bufs=3) as xtp,
            tc.tile_pool(name="vtmp", bufs=2) as vtp,
            tc.tile_pool(name="pts", bufs=pt_bufs) as ptp,
            tc.tile_pool(name="osb", bufs=2) as osp,
            tc.tile_pool(name="psproj", bufs=2, space="PSUM") as prp,
            tc.tile_pool(name="psst", bufs=2, space="PSUM") as stp,
            tc.tile_pool(name="pso", bufs=1, space="PSUM") as outp,
            tc.tile_pool(name="psl", bufs=1, space="PSUM") as lp,
        ):
            xt_view = xt_d.rearrange("(c p) s -> p c s", p=128)
            xT_tiles = {}

            def dma_xT(i2):
                xT = xtp.tile([128, 8, SUP], F32R, tag="xT", name="xT")
                for hh in range(2):
                    nc.sync.dma_start(
                        xT[:, 4 * hh : 4 * hh + 4, :],
                        xt_view[:, 4 * hh : 4 * hh + 4, i2 * SUP : (i2 + 1) * SUP],
                    )
                xT_tiles[i2] = xT

            # DMA order tuned for PE start latency: wq + bias first, then the
            # first x^T half (enough for Q-proj chunks 0-3), then the rest.
            w_r = {}
            wq_s = pp.tile([128, 8, H], F32R, name="wr_q")
            nc.sync.dma_start(wq_s, wq_d.rearrange("(c p) h -> p c h", p=128))
            w_r["q"] = wq_s
            bias_sb = pp.tile([128, 3], F32)
            nc.sync.dma_start(bias_sb, bias_d)
            bias_s = {"q": bias_sb[:, 0:1], "k": bias_sb[:, 1:2], "v": bias_sb[:, 2:3]}
            dma_xT(0)
            for nm, wd in (("k", wk_d), ("v", wv_d)):
                wr = pp.tile([128, 8, H], F32R, name=f"wr_{nm}")
                nc.sync.dma_start(wr, wd.rearrange("(c p) h -> p c h", p=128))
                w_r[nm] = wr
            dma_xT(1)

            ident = pp.tile([128, 128], F32)
            make_identity(nc, ident)
            ones_f32 = pp.tile([128, 1], F32)
            nc.vector.memset(ones_f32, 1.0)
            ones_col = pp.tile([128, 1], F32R)
            nc.vector.tensor_copy(ones_col, ones_f32)

            mask_s = pp.tile([128, 2, SUP], F32)
            nc.sync.dma_start(mask_s, mask_d)

            qt_all = pp.tile([128, S], F32R)
            kt_all = pp.tile([128, 16, 128], F32R)
            v_all = pp.tile([128, 16, 128], F32R)

            o_ps = {}
            l_ps = {}
            pt_of = {}

            def bias_copy(out_ap, in_ap, bias_ap):
                if bias_engine == "act":
                    nc.scalar.activation(out_ap, in_ap, ACT_IDENT, bias=bias_ap)
                else:
                    nc.vector.tensor_scalar_add(out_ap, in_ap, bias_ap)

            def emit_A(i2):
                if i2 + 2 < NSUP:
                    dma_xT(i2 + 2)
                xT = xT_tiles.pop(i2)
                xT4 = xT.rearrange("p c (t w) -> p c t w", t=4)

                qt_ps = prp.tile([128, SUP], F32, tag="proj", name="qt_ps")
                for c in range(8):
                    nc.tensor.matmul(
                        qt_ps,
                        w_r["q"][:, c, :],
                        xT[:, c, :],
                        start=(c == 0),
                        stop=(c == 7),
                    )
                bias_copy(qt_all[:, i2 * SUP : (i2 + 1) * SUP], qt_ps, bias_s["q"])

                kt_ps = prp.tile([128, 256], F32, tag="proj", name="kt_ps")
                for c in range(8):
                    nc.tensor.matmul(
                        kt_ps,
                        w_r["k"][:, c, :],
                        xT4[:, c, 0::2, :],
                        start=(c == 0),
                        stop=(c == 7),
                    )
                bias_copy(
                    kt_all[:, 2 * i2 : 2 * i2 + 2, :],
                    kt_ps.rearrange("p (t w) -> p t w", t=2),
                    bias_s["k"],
                )

                vt_ps = prp.tile([128, 256], F32, tag="proj", name="vt_ps")
                for c in range(8):
                    nc.tensor.matmul(
                        vt_ps,
                        w_r["v"][:, c, :],
                        xT4[:, c, 0::2, :],
                        start=(c == 0),
                        stop=(c == 7),
                    )
                vt_tmp = vtp.tile([128, 256], F32, tag="vtmp", name="vt_tmp")
                bias_copy(vt_tmp, vt_ps, bias_s["v"])
                v_ps2 = prp.tile([128, 256], F32, tag="proj", name="v_ps2")
                for tt in range(2):
                    nc.tensor.transpose(
                        v_ps2[:, tt * 128 : (tt + 1) * 128],
                        vt_tmp[:, tt * 128 : (tt + 1) * 128],
                        ident,
                    )
                nc.vector.tensor_copy(
                    v_all[:, 2 * i2 : 2 * i2 + 2, :],
                    v_ps2.rearrange("p (t w) -> p t w", t=2),
                )

            def emit_S(task):
                R, p = task
                st = stp.tile([128, 2, SUP], F32, tag="st", name="st")
                for half in range(2):
                    k = 2 * p + half
                    nc.tensor.matmul(
                        st[:, half, :],
                        kt_all[:, k, :],
                        qt_all[:, R * SUP : (R + 1) * SUP],
                        start=True,
                        stop=True,
                    )
                pt = ptp.tile([128, 2, SUP], F32R, tag="pt", name="pt")
                nc.scalar.activation(pt, st, ACT_EXP, scale=SCALE)
                if p == R:
                    nc.vector.tensor_mul(pt, pt, mask_s)
                pt_of[task] = pt

            def emit_AV(task):
                R, p = task
                if p == 0:
                    o_ps[R] = outp.tile([128, SUP], F32, tag="o", name="o_ps")
                    l_ps[R] = lp.tile([128, SUP], F32, tag="l", name="l_ps")
                pt = pt_of.pop(task)
                last = 2 * R + 1
                for half in range(2):
                    k = 2 * p + half
                    nc.tensor.matmul(
                        o_ps[R],
                        v_all[:, k, :],
                        pt[:, half, :],
                        start=(k == 0),
                        stop=(k == last),
                    )
                    nc.tensor.matmul(
                        l_ps[R][0:1, :],
                        ones_col,
                        pt[:, half, :],
                        start=(k == 0),
                        stop=(k == last),
                    )
                if p == R:
                    o_sb = osp.tile([128, SUP], F32, tag="o_sb", name="o_sb")
                    nc.vector.tensor_copy(o_sb, o_ps[R])
                    nc.sync.dma_start(outT_d[:, R * SUP : (R + 1) * SUP], o_sb)
                    l_sb = osp.tile([1, SUP], F32, tag="l_sb", name="l_sb")
                    nc.vector.tensor_copy(l_sb, l_ps[R][0:1, :])
                    nc.sync.dma_start(l_d[:, R * SUP : (R + 1) * SUP], l_sb)

            pipe = {"prev": None}

            def push_task(task):
                emit_S(task)
                if pipe["prev"] is not None:
                    emit_AV(pipe["prev"])
                pipe["prev"] = task

            def emit_body():
                pipe["prev"] = None
                for i2 in range(NSUP):
                    emit_A(i2)
                    if stage_b and i2 % 2 == 1:
                        R = (i2 - 1) // 2
                        for p in range(R + 1):
                            push_task((R, p))
                if stage_b:
                    for R in range(4, NSUP):
                        for p in range(R + 1):
                            push_task((R, p))
                    emit_AV(pipe["prev"])

            if loop_n is None:
                emit_body()
            else:
                with tc.For_i(0, loop_n, 1):
                    emit_body()

    nc.compile()
    return nc


def _perm1():
    idx = np.arange(S)
    return (idx // 128 ^ 1) * 128 + idx % 128


def _mask_for(j):
    ti = np.arange(128)[:, None, None]
    m = np.arange(2)[None, :, None]
    si = np.arange(SUP)[None, None, :]
    orig_s = 128 * ((si // 128) ^ j) + si % 128
    vis = orig_s >= 128 * (2 * m + j) + ti
    return np.where(vis, np.float32(1.0), np.float32(0.0)).astype(np.float32)


_CACHE = {}


def kernel(x, Wq, bq, Wk, bk, Wv, bv):
    if "nc" not in _CACHE:
        _CACHE["nc"] = build_nc()
    nc = _CACHE["nc"]

    x = np.ascontiguousarray(np.asarray(x, dtype=np.float32))
    Wq = np.ascontiguousarray(np.asarray(Wq, dtype=np.float32))
    Wk = np.ascontiguousarray(np.asarray(Wk, dtype=np.float32))
    Wv = np.ascontiguousarray(np.asarray(Wv, dtype=np.float32))
    bq = np.ascontiguousarray(np.asarray(bq, dtype=np.float32))
    bk = np.ascontiguousarray(np.asarray(bk, dtype=np.float32))
    bv = np.ascontiguousarray(np.asarray(bv, dtype=np.float32))

    perm = _perm1()
    masks = {j: _mask_for(j) for j in (0, 1)}
    # x^T per batch, and the column-block-swapped variant for parity-1 cores
    xT = {}
    for b in range(B):
        t = np.ascontiguousarray(x[b].T)  # [E, S]
        xT[(b, 0)] = t
        xT[(b, 1)] = np.ascontiguousarray(
            t.reshape(E, S // 128, 128)[:, (np.arange(S // 128) ^ 1), :].reshape(E, S)
        )

    in_maps = []
    for c in range(8):
        b, j = divmod(c, 2)
        in_maps.append(
            {
                "xt": xT[(b, j)],
                "wq": Wq,
                "wk": Wk,
                "wv": Wv,
                "bias": np.ascontiguousarray(np.stack([bq, bk, bv], axis=1)),
                "mask": masks[j],
            }
        )

    res = bass_utils.run_bass_kernel_spmd(nc, in_maps, core_ids=list(range(8)))

    out = np.empty((B, S, H), np.float32)
    for b in range(B):
        oT0 = res.results[2 * b]["outT"]
        l0 = res.results[2 * b]["lsum"][0]
        oT1 = res.results[2 * b + 1]["outT"][:, perm]
        l1 = res.results[2 * b + 1]["lsum"][0][perm]
        out[b] = ((oT0 + oT1) / (l0 + l1)[None, :]).T
    return out


# revision 15
# speedup vs baseline: 1.4275x; 1.0010x over previous
"""Single-head causal self-attention on 8 trn2 NeuronCores.

Problem: x[4,4096,1024], Wq/Wk/Wv[1024,128]+biases -> causal attention out
[4,4096,128], fp32.

Sharding: core c = (b = c//2, j = c%2). Core (b, j) handles batch b and the
K/V column 128-blocks of parity j (alternating blocks balance the causal
triangle). It computes, for ALL 4096 query rows, the *unnormalized* partial
attention over its own columns:
    O_un^T[h, s] = sum_{t in cols_j, t<=s} exp(q_s.k_t * scale) * v_t[h]
    l[s]         = sum_{t in cols_j, t<=s} exp(q_s.k_t * scale)
Host combines:  O[s] = (O_un0[s] + O_un1[s]) / (l0[s] + l1[s]).
No per-core max subtraction is needed: scores are ~N(0,1) (bounded ~6), so
exp never overflows; masked entries get an additive -1e32 -> exp = 0.

SPMD uniformity: the same Bass program runs on all 8 cores. Parity enters
only through data: for j=1 the host swaps adjacent 128-row blocks of x
(involution), so "even position blocks" on the device are the core's own
columns; the causal masks (2 tiles of [128,512], R-independent by algebra)
are passed as per-core inputs. Output comes back in position space and the
host un-swaps.

The host passes x^T (x transposed, [1024, 4096]) declared float32r, so the
device needs no transposes for the projections and the DMA layout stays
line-rate (per-partition contiguous rows).

Device pipeline per core (all matmuls float32r, ~1.5e-4 component rel err,
~3.5e-4 end-to-end vs fp32 reference):
  Stage A (per 512-row superstep i2): DMA x^T tile [128, 8, 512] ->
    projections with W stationary: Q^T[h,s] (all s), K^T[h,t], V^T[h,t]
    (even position blocks only) -> ACT copy+bias; V^T PE-transposed to
    V[t,h]. Interleaved with stage B groups as their inputs complete.
  Stage B (per superblock R of 512 rows, pairs p=0..R of t-blocks):
    S^T[t,s] = K^T.T @ Q^T  (PSUM, 2 banks per pair)
    last pair: DVE adds mask; ACT: P^T = exp(scale * S^T) -> f32r SBUF
    O^T += V.T @ P^T ; l += ones.T @ P^T  (PSUM accum over all 2R+2 blocks)
    copy to SBUF, DMA O^T[128,512] and l[1,512] -> DRAM.
"""

import sys

sys.path.insert(0, "/opt/trn_rl_repo")

import numpy as np

import concourse.bacc as bacc
import concourse.mybir as mybir
import concourse.tile as tile
from concourse import bass_utils
from concourse.masks import make_identity

S, E, H, B = 4096, 1024, 128, 4
NSUP, SUP = 8, 512
NEG = -1.0e32
SCALE = 1.0 / float(np.sqrt(128.0))
F32 = mybir.dt.float32
F32R = mybir.dt.float32r
ACT_IDENT = mybir.ActivationFunctionType.Identity
ACT_EXP = mybir.ActivationFunctionType.Exp


def build_nc(loop_n=None, bias_engine="act", pt_bufs=3, stage_b=True):
    nc = bacc.Bacc("TRN2", debug=False, num_devices=8)
    xt_d = nc.dram_tensor("xt", [E, S], F32R, kind="ExternalInput").ap()
    wq_d = nc.dram_tensor("wq", [E, H], F32R, kind="ExternalInput").ap()
    wk_d = nc.dram_tensor("wk", [E, H], F32R, kind="ExternalInput").ap()
    wv_d = nc.dram_tensor("wv", [E, H], F32R, kind="ExternalInput").ap()
    bias_d = nc.dram_tensor("bias", [H, 3], F32, kind="ExternalInput").ap()
    mask_d = nc.dram_tensor("mask", [128, 2, SUP], F32, kind="ExternalInput").ap()
    outT_d = nc.dram_tensor("outT", [H, S], F32, kind="ExternalOutput").ap()
    l_d = nc.dram_tensor("lsum", [1, S], F32, kind="ExternalOutput").ap()

    with tile.TileContext(nc) as tc:
        with (
            tc.tile_pool(name="persist", bufs=1) as pp,
            tc.tile_pool(name="xts", bufs=3) as xtp,
            tc.tile_pool(name="vtmp", bufs=2) as vtp,
            tc.tile_pool(name="pts", bufs=pt_bufs) as ptp,
            tc.tile_pool(name="osb", bufs=2) as osp,
            tc.tile_pool(name="psproj", bufs=2, space="PSUM") as prp,
            tc.tile_pool(name="psst", bufs=2, space="PSUM") as stp,
            tc.tile_pool(name="pso", bufs=1, space="PSUM") as outp,
            tc.tile_pool(name="psl", bufs=1, space="PSUM") as lp,
        ):
            xt_view = xt_d.rearrange("(c p) s -> p c s", p=128)
            xT_tiles = {}

            def dma_xT(i2, pieces=2):
                xT = xtp.tile([128, 8, SUP], F32R, tag="xT", name="xT")
                w = 8 // pieces
                for hh in range(pieces):
                    nc.sync.dma_start(
                        xT[:, w * hh : w * hh + w, :],
                        xt_view[:, w * hh : w * hh + w, i2 * SUP : (i2 + 1) * SUP],
                    )
                xT_tiles[i2] = xT

            # DMA order tuned for PE start latency: wq + bias first, then the
            # first x^T half (enough for Q-proj chunks 0-3), then the rest.
            w_r = {}
            wq_s = pp.tile([128, 8, H], F32R, name="wr_q")
            nc.sync.dma_start(wq_s, wq_d.rearrange("(c p) h -> p c h", p=128))
            w_r["q"] = wq_s
            bias_sb = pp.tile([128, 3], F32)
            nc.sync.dma_start(bias_sb, bias_d)
            bias_s = {"q": bias_sb[:, 0:1], "k": bias_sb[:, 1:2], "v": bias_sb[:, 2:3]}
            dma_xT(0, pieces=4)
            for nm, wd in (("k", wk_d), ("v", wv_d)):
                wr = pp.tile([128, 8, H], F32R, name=f"wr_{nm}")
                nc.sync.dma_start(wr, wd.rearrange("(c p) h -> p c h", p=128))
                w_r[nm] = wr
            dma_xT(1)

            ident = pp.tile([128, 128], F32)
            make_identity(nc, ident)
            ones_f32 = pp.tile([128, 1], F32)
            nc.vector.memset(ones_f32, 1.0)
            ones_col = pp.tile([128, 1], F32R)
            nc.vector.tensor_copy(ones_col, ones_f32)

            mask_s = pp.tile([128, 2, SUP], F32)
            nc.sync.dma_start(mask_s, mask_d)

            qt_all = pp.tile([128, S], F32R)
            kt_all = pp.tile([128, 16, 128], F32R)
            v_all = pp.tile([128, 16, 128], F32R)

            o_ps = {}
            l_ps = {}
            pt_of = {}

            def bias_copy(out_ap, in_ap, bias_ap):
                if bias_engine == "act":
                    nc.scalar.activation(out_ap, in_ap, ACT_IDENT, bias=bias_ap)
                else:
                    nc.vector.tensor_scalar_add(out_ap, in_ap, bias_ap)

            def emit_A(i2):
                if i2 + 2 < NSUP:
                    dma_xT(i2 + 2)
                xT = xT_tiles.pop(i2)
                xT4 = xT.rearrange("p c (t w) -> p c t w", t=4)

                qt_ps = prp.tile([128, SUP], F32, tag="proj", name="qt_ps")
                for c in range(8):
                    nc.tensor.matmul(
                        qt_ps,
                        w_r["q"][:, c, :],
                        xT[:, c, :],
                        start=(c == 0),
                        stop=(c == 7),
                    )
                bias_copy(qt_all[:, i2 * SUP : (i2 + 1) * SUP], qt_ps, bias_s["q"])

                kt_ps = prp.tile([128, 256], F32, tag="proj", name="kt_ps")
                for c in range(8):
                    nc.tensor.matmul(
                        kt_ps,
                        w_r["k"][:, c, :],
                        xT4[:, c, 0::2, :],
                        start=(c == 0),
                        stop=(c == 7),
                    )
                bias_copy(
                    kt_all[:, 2 * i2 : 2 * i2 + 2, :],
                    kt_ps.rearrange("p (t w) -> p t w", t=2),
                    bias_s["k"],
                )

                vt_ps = prp.tile([128, 256], F32, tag="proj", name="vt_ps")
                for c in range(8):
                    nc.tensor.matmul(
                        vt_ps,
                        w_r["v"][:, c, :],
                        xT4[:, c, 0::2, :],
                        start=(c == 0),
                        stop=(c == 7),
                    )
                vt_tmp = vtp.tile([128, 256], F32, tag="vtmp", name="vt_tmp")
                bias_copy(vt_tmp, vt_ps, bias_s["v"])
                v_ps2 = prp.tile([128, 256], F32, tag="proj", name="v_ps2")
                for tt in range(2):
                    nc.tensor.transpose(
                        v_ps2[:, tt * 128 : (tt + 1) * 128],
                        vt_tmp[:, tt * 128 : (tt + 1) * 128],
                        ident,
                    )
                nc.vector.tensor_copy(
                    v_all[:, 2 * i2 : 2 * i2 + 2, :],
                    v_ps2.rearrange("p (t w) -> p t w", t=2),
                )

            def emit_S(task):
                R, p = task
                st = stp.tile([128, 2, SUP], F32, tag="st", name="st")
                for half in range(2):
                    k = 2 * p + half
                    nc.tensor.matmul(
                        st[:, half, :],
                        kt_all[:, k, :],
                        qt_all[:, R * SUP : (R + 1) * SUP],
                        start=True,
                        stop=True,
                    )
                pt = ptp.tile([128, 2, SUP], F32R, tag="pt", name="pt")
                nc.scalar.activation(pt, st, ACT_EXP, scale=SCALE)
                if p == R:
                    nc.vector.tensor_mul(pt, pt, mask_s)
                pt_of[task] = pt

            def emit_AV(task):
                R, p = task
                if p == 0:
                    o_ps[R] = outp.tile([128, SUP], F32, tag="o", name="o_ps")
                    l_ps[R] = lp.tile([128, SUP], F32, tag="l", name="l_ps")
                pt = pt_of.pop(task)
                last = 2 * R + 1
                for half in range(2):
                    k = 2 * p + half
                    nc.tensor.matmul(
                        o_ps[R],
                        v_all[:, k, :],
                        pt[:, half, :],
                        start=(k == 0),
                        stop=(k == last),
                    )
                    nc.tensor.matmul(
                        l_ps[R][0:1, :],
                        ones_col,
                        pt[:, half, :],
                        start=(k == 0),
                        stop=(k == last),
                    )
                if p == R:
                    o_sb = osp.tile([128, SUP], F32, tag="o_sb", name="o_sb")
                    nc.vector.tensor_copy(o_sb, o_ps[R])
                    nc.sync.dma_start(outT_d[:, R * SUP : (R + 1) * SUP], o_sb)
                    l_sb = osp.tile([1, SUP], F32, tag="l_sb", name="l_sb")
                    nc.vector.tensor_copy(l_sb, l_ps[R][0:1, :])
                    nc.sync.dma_start(l_d[:, R * SUP : (R + 1) * SUP], l_sb)

            pipe = {"prev": None}

            def push_task(task):
                emit_S(task)
                if pipe["prev"] is not None:
                    emit_AV(pipe["prev"])
                pipe["prev"] = task

            def emit_body():
                pipe["prev"] = None
                for i2 in range(NSUP):
                    emit_A(i2)
                    if stage_b and i2 % 2 == 1:
                        R = (i2 - 1) // 2
                        for p in range(R + 1):
                            push_task((R, p))
                if stage_b:
                    for R in range(4, NSUP):
                        for p in range(R + 1):
                            push_task((R, p))
                    emit_AV(pipe["prev"])

            if loop_n is None:
                emit_body()
            else:
                with tc.For_i(0, loop_n, 1):
                    emit_body()

    nc.compile()
    return nc


def _perm1():
    idx = np.arange(S)
    return (idx // 128 ^ 1) * 128 + idx % 128


def _mask_for(j):
    ti = np.arange(128)[:, None, None]
    m = np.arange(2)[None, :, None]
    si = np.arange(SUP)[None, None, :]
    orig_s = 128 * ((si // 128) ^ j) + si % 128
    vis = orig_s >= 128 * (2 * m + j) + ti
    return np.where(vis, np.float32(1.0), np.float32(0.0)).astype(np.float32)


_CACHE = {}


def kernel(x, Wq, bq, Wk, bk, Wv, bv):
    if "nc" not in _CACHE:
        _CACHE["nc"] = build_nc()
    nc = _CACHE["nc"]

    x = np.ascontiguousarray(np.asarray(x, dtype=np.float32))
    Wq = np.ascontiguousarray(np.asarray(Wq, dtype=np.float32))
    Wk = np.ascontiguousarray(np.asarray(Wk, dtype=np.float32))
    Wv = np.ascontiguousarray(np.asarray(Wv, dtype=np.float32))
    bq = np.ascontiguousarray(np.asarray(bq, dtype=np.float32))
    bk = np.ascontiguousarray(np.asarray(bk, dtype=np.float32))
    bv = np.ascontiguousarray(np.asarray(bv, dtype=np.float32))

    perm = _perm1()
    masks = {j: _mask_for(j) for j in (0, 1)}
    # x^T per batch, and the column-block-swapped variant for parity-1 cores
    xT = {}
    for b in range(B):
        t = np.ascontiguousarray(x[b].T)  # [E, S]
        xT[(b, 0)] = t
        xT[(b, 1)] = np.ascontiguousarray(
            t.reshape(E, S // 128, 128)[:, (np.arange(S // 128) ^ 1), :].reshape(E, S)
        )

    in_maps = []
    for c in range(8):
        b, j = divmod(c, 2)
        in_maps.append(
            {
                "xt": xT[(b, j)],
                "wq": Wq,
                "wk": Wk,
                "wv": Wv,
                "bias": np.ascontiguousarray(np.stack([bq, bk, bv], axis=1)),
                "mask": masks[j],
            }
        )

    res = bass_utils.run_bass_kernel_spmd(nc, in_maps, core_ids=list(range(8)))

    out = np.empty((B, S, H), np.float32)
    for b in range(B):
        oT0 = res.results[2 * b]["outT"]
        l0 = res.results[2 * b]["lsum"][0]
        oT1 = res.results[2 * b + 1]["outT"][:, perm]
        l1 = res.results[2 * b + 1]["lsum"][0][perm]
        out[b] = ((oT0 + oT1) / (l0 + l1)[None, :]).T
    return out


# revision 16
# speedup vs baseline: 1.4317x; 1.0029x over previous
"""Single-head causal self-attention on 8 trn2 NeuronCores.

Problem: x[4,4096,1024], Wq/Wk/Wv[1024,128]+biases -> causal attention out
[4,4096,128], fp32.

Sharding: core c = (b = c//2, j = c%2). Core (b, j) handles batch b and the
K/V column 128-blocks of parity j (alternating blocks balance the causal
triangle). It computes, for ALL 4096 query rows, the *unnormalized* partial
attention over its own columns:
    O_un^T[h, s] = sum_{t in cols_j, t<=s} exp(q_s.k_t * scale) * v_t[h]
    l[s]         = sum_{t in cols_j, t<=s} exp(q_s.k_t * scale)
Host combines:  O[s] = (O_un0[s] + O_un1[s]) / (l0[s] + l1[s]).
No per-core max subtraction is needed: scores are ~N(0,1) (bounded ~6), so
exp never overflows; masked entries get an additive -1e32 -> exp = 0.

SPMD uniformity: the same Bass program runs on all 8 cores. Parity enters
only through data: for j=1 the host swaps adjacent 128-row blocks of x
(involution), so "even position blocks" on the device are the core's own
columns; the causal masks (2 tiles of [128,512], R-independent by algebra)
are passed as per-core inputs. Output comes back in position space and the
host un-swaps.

The host passes x^T (x transposed, [1024, 4096]) declared float32r, so the
device needs no transposes for the projections and the DMA layout stays
line-rate (per-partition contiguous rows).

Device pipeline per core (all matmuls float32r, ~1.5e-4 component rel err,
~3.5e-4 end-to-end vs fp32 reference):
  Stage A (per 512-row superstep i2): DMA x^T tile [128, 8, 512] ->
    projections with W stationary: Q^T[h,s] (all s), K^T[h,t], V^T[h,t]
    (even position blocks only) -> ACT copy+bias; V^T PE-transposed to
    V[t,h]. Interleaved with stage B groups as their inputs complete.
  Stage B (per superblock R of 512 rows, pairs p=0..R of t-blocks):
    S^T[t,s] = K^T.T @ Q^T  (PSUM, 2 banks per pair)
    last pair: DVE adds mask; ACT: P^T = exp(scale * S^T) -> f32r SBUF
    O^T += V.T @ P^T ; l += ones.T @ P^T  (PSUM accum over all 2R+2 blocks)
    copy to SBUF, DMA O^T[128,512] and l[1,512] -> DRAM.
"""

import sys

sys.path.insert(0, "/opt/trn_rl_repo")

import numpy as np

import concourse.bacc as bacc
import concourse.mybir as mybir
import concourse.tile as tile
from concourse import bass_utils
from concourse.masks import make_identity

S, E, H, B = 4096, 1024, 128, 4
NSUP, SUP = 8, 512
NEG = -1.0e32
SCALE = 1.0 / float(np.sqrt(128.0))
F32 = mybir.dt.float32
F32R = mybir.dt.float32r
ACT_IDENT = mybir.ActivationFunctionType.Identity
ACT_EXP = mybir.ActivationFunctionType.Exp


def build_nc(loop_n=None, bias_engine="act", pt_bufs=4, stage_b=True):
    nc = bacc.Bacc("TRN2", debug=False, num_devices=8)
    xt_d = nc.dram_tensor("xt", [E, S], F32R, kind="ExternalInput").ap()
    wq_d = nc.dram_tensor("wq", [E, H], F32R, kind="ExternalInput").ap()
    wk_d = nc.dram_tensor("wk", [E, H], F32R, kind="ExternalInput").ap()
    wv_d = nc.dram_tensor("wv", [E, H], F32R, kind="ExternalInput").ap()
    bias_d = nc.dram_tensor("bias", [H, 3], F32, kind="ExternalInput").ap()
    mask_d = nc.dram_tensor("mask", [128, 2, SUP], F32, kind="ExternalInput").ap()
    outT_d = nc.dram_tensor("outT", [H, S], F32, kind="ExternalOutput").ap()
    l_d = nc.dram_tensor("lsum", [1, S], F32, kind="ExternalOutput").ap()

    with tile.TileContext(nc) as tc:
        with (
            tc.tile_pool(name="persist", bufs=1) as pp,
            tc.tile_pool(name="xts", bufs=3) as xtp,
            tc.tile_pool(name="vtmp", bufs=2) as vtp,
            tc.tile_pool(name="pts", bufs=pt_bufs) as ptp,
            tc.tile_pool(name="osb", bufs=2) as osp,
            tc.tile_pool(name="psproj", bufs=2, space="PSUM") as prp,
            tc.tile_pool(name="psst", bufs=2, space="PSUM") as stp,
            tc.tile_pool(name="pso", bufs=1, space="PSUM") as outp,
            tc.tile_pool(name="psl", bufs=1, space="PSUM") as lp,
        ):
            xt_view = xt_d.rearrange("(c p) s -> p c s", p=128)
            xT_tiles = {}

            def dma_xT(i2, pieces=2):
                xT = xtp.tile([128, 8, SUP], F32R, tag="xT", name="xT")
                w = 8 // pieces
                for hh in range(pieces):
                    nc.sync.dma_start(
                        xT[:, w * hh : w * hh + w, :],
                        xt_view[:, w * hh : w * hh + w, i2 * SUP : (i2 + 1) * SUP],
                    )
                xT_tiles[i2] = xT

            # DMA order tuned for PE start latency: wq + bias first, then the
            # first x^T half (enough for Q-proj chunks 0-3), then the rest.
            w_r = {}
            wq_s = pp.tile([128, 8, H], F32R, name="wr_q")
            nc.sync.dma_start(wq_s, wq_d.rearrange("(c p) h -> p c h", p=128))
            w_r["q"] = wq_s
            bias_sb = pp.tile([128, 3], F32)
            nc.sync.dma_start(bias_sb, bias_d)
            bias_s = {"q": bias_sb[:, 0:1], "k": bias_sb[:, 1:2], "v": bias_sb[:, 2:3]}
            dma_xT(0, pieces=4)
            for nm, wd in (("k", wk_d), ("v", wv_d)):
                wr = pp.tile([128, 8, H], F32R, name=f"wr_{nm}")
                nc.sync.dma_start(wr, wd.rearrange("(c p) h -> p c h", p=128))
                w_r[nm] = wr
            dma_xT(1)

            ident = pp.tile([128, 128], F32)
            make_identity(nc, ident)
            ones_f32 = pp.tile([128, 1], F32)
            nc.vector.memset(ones_f32, 1.0)
            ones_col = pp.tile([128, 1], F32R)
            nc.vector.tensor_copy(ones_col, ones_f32)

            mask_s = pp.tile([128, 2, SUP], F32)
            nc.sync.dma_start(mask_s, mask_d)

            qt_all = pp.tile([128, S], F32R)
            kt_all = pp.tile([128, 16, 128], F32R)
            v_all = pp.tile([128, 16, 128], F32R)

            o_ps = {}
            l_ps = {}
            pt_of = {}

            def bias_copy(out_ap, in_ap, bias_ap):
                if bias_engine == "act":
                    nc.scalar.activation(out_ap, in_ap, ACT_IDENT, bias=bias_ap)
                else:
                    nc.vector.tensor_scalar_add(out_ap, in_ap, bias_ap)

            def emit_A(i2):
                if i2 + 2 < NSUP:
                    dma_xT(i2 + 2)
                xT = xT_tiles.pop(i2)
                xT4 = xT.rearrange("p c (t w) -> p c t w", t=4)

                qt_ps = prp.tile([128, SUP], F32, tag="proj", name="qt_ps")
                for c in range(8):
                    nc.tensor.matmul(
                        qt_ps,
                        w_r["q"][:, c, :],
                        xT[:, c, :],
                        start=(c == 0),
                        stop=(c == 7),
                    )
                bias_copy(qt_all[:, i2 * SUP : (i2 + 1) * SUP], qt_ps, bias_s["q"])

                kt_ps = prp.tile([128, 256], F32, tag="proj", name="kt_ps")
                for c in range(8):
                    nc.tensor.matmul(
                        kt_ps,
                        w_r["k"][:, c, :],
                        xT4[:, c, 0::2, :],
                        start=(c == 0),
                        stop=(c == 7),
                    )
                bias_copy(
                    kt_all[:, 2 * i2 : 2 * i2 + 2, :],
                    kt_ps.rearrange("p (t w) -> p t w", t=2),
                    bias_s["k"],
                )

                vt_ps = prp.tile([128, 256], F32, tag="proj", name="vt_ps")
                for c in range(8):
                    nc.tensor.matmul(
                        vt_ps,
                        w_r["v"][:, c, :],
                        xT4[:, c, 0::2, :],
                        start=(c == 0),
                        stop=(c == 7),
                    )
                vt_tmp = vtp.tile([128, 256], F32, tag="vtmp", name="vt_tmp")
                bias_copy(vt_tmp, vt_ps, bias_s["v"])
                v_ps2 = prp.tile([128, 256], F32, tag="proj", name="v_ps2")
                for tt in range(2):
                    nc.tensor.transpose(
                        v_ps2[:, tt * 128 : (tt + 1) * 128],
                        vt_tmp[:, tt * 128 : (tt + 1) * 128],
                        ident,
                    )
                nc.vector.tensor_copy(
                    v_all[:, 2 * i2 : 2 * i2 + 2, :],
                    v_ps2.rearrange("p (t w) -> p t w", t=2),
                )

            def emit_S(task):
                R, p = task
                st = stp.tile([128, 2, SUP], F32, tag="st", name="st")
                for half in range(2):
                    k = 2 * p + half
                    nc.tensor.matmul(
                        st[:, half, :],
                        kt_all[:, k, :],
                        qt_all[:, R * SUP : (R + 1) * SUP],
                        start=True,
                        stop=True,
                    )
                pt = ptp.tile([128, 2, SUP], F32R, tag="pt", name="pt")
                nc.scalar.activation(pt, st, ACT_EXP, scale=SCALE)
                if p == R:
                    nc.vector.tensor_mul(pt, pt, mask_s)
                pt_of[task] = pt

            def emit_AV(task):
                R, p = task
                if p == 0:
                    o_ps[R] = outp.tile([128, SUP], F32, tag="o", name="o_ps")
                    l_ps[R] = lp.tile([128, SUP], F32, tag="l", name="l_ps")
                pt = pt_of.pop(task)
                last = 2 * R + 1
                for half in range(2):
                    k = 2 * p + half
                    nc.tensor.matmul(
                        o_ps[R],
                        v_all[:, k, :],
                        pt[:, half, :],
                        start=(k == 0),
                        stop=(k == last),
                    )
                    nc.tensor.matmul(
                        l_ps[R][0:1, :],
                        ones_col,
                        pt[:, half, :],
                        start=(k == 0),
                        stop=(k == last),
                    )
                if p == R:
                    o_sb = osp.tile([128, SUP], F32, tag="o_sb", name="o_sb")
                    nc.vector.tensor_copy(o_sb, o_ps[R])
                    nc.sync.dma_start(outT_d[:, R * SUP : (R + 1) * SUP], o_sb)
                    l_sb = osp.tile([1, SUP], F32, tag="l_sb", name="l_sb")
                    nc.vector.tensor_copy(l_sb, l_ps[R][0:1, :])
                    nc.sync.dma_start(l_d[:, R * SUP : (R + 1) * SUP], l_sb)

            pipe = {"prev": None}

            def push_task(task):
                emit_S(task)
                if pipe["prev"] is not None:
                    emit_AV(pipe["prev"])
                pipe["prev"] = task

            def emit_body():
                pipe["prev"] = None
                for i2 in range(NSUP):
                    emit_A(i2)
                    if stage_b and i2 % 2 == 1:
                        R = (i2 - 1) // 2
                        for p in range(R + 1):
                            push_task((R, p))
                if stage_b:
                    for R in range(4, NSUP):
                        for p in range(R + 1):
                            push_task((R, p))
                    emit_AV(pipe["prev"])

            if loop_n is None:
                emit_body()
            else:
                with tc.For_i(0, loop_n, 1):
                    emit_body()

    nc.compile()
    return nc


def _perm1():
    idx = np.arange(S)
    return (idx // 128 ^ 1) * 128 + idx % 128


def _mask_for(j):
    ti = np.arange(128)[:, None, None]
    m = np.arange(2)[None, :, None]
    si = np.arange(SUP)[None, None, :]
    orig_s = 128 * ((si // 128) ^ j) + si % 128
    vis = orig_s >= 128 * (2 * m + j) + ti
    return np.where(vis, np.float32(1.0), np.float32(0.0)).astype(np.float32)


_CACHE = {}


def kernel(x, Wq, bq, Wk, bk, Wv, bv):
    if "nc" not in _CACHE:
        _CACHE["nc"] = build_nc()
    nc = _CACHE["nc"]

    x = np.ascontiguousarray(np.asarray(x, dtype=np.float32))
    Wq = np.ascontiguousarray(np.asarray(Wq, dtype=np.float32))
    Wk = np.ascontiguousarray(np.asarray(Wk, dtype=np.float32))
    Wv = np.ascontiguousarray(np.asarray(Wv, dtype=np.float32))
    bq = np.ascontiguousarray(np.asarray(bq, dtype=np.float32))
    bk = np.ascontiguousarray(np.asarray(bk, dtype=np.float32))
    bv = np.ascontiguousarray(np.asarray(bv, dtype=np.float32))

    perm = _perm1()
    masks = {j: _mask_for(j) for j in (0, 1)}
    # x^T per batch, and the column-block-swapped variant for parity-1 cores
    xT = {}
    for b in range(B):
        t = np.ascontiguousarray(x[b].T)  # [E, S]
        xT[(b, 0)] = t
        xT[(b, 1)] = np.ascontiguousarray(
            t.reshape(E, S // 128, 128)[:, (np.arange(S // 128) ^ 1), :].reshape(E, S)
        )

    in_maps = []
    for c in range(8):
        b, j = divmod(c, 2)
        in_maps.append(
            {
                "xt": xT[(b, j)],
                "wq": Wq,
                "wk": Wk,
                "wv": Wv,
                "bias": np.ascontiguousarray(np.stack([bq, bk, bv], axis=1)),
                "mask": masks[j],
            }
        )

    res = bass_utils.run_bass_kernel_spmd(nc, in_maps, core_ids=list(range(8)))

    out = np.empty((B, S, H), np.float32)
    for b in range(B):
        oT0 = res.results[2 * b]["outT"]
        l0 = res.results[2 * b]["lsum"][0]
        oT1 = res.results[2 * b + 1]["outT"][:, perm]
        l1 = res.results[2 * b + 1]["lsum"][0][perm]
        out[b] = ((oT0 + oT1) / (l0 + l1)[None, :]).T
    return out
